# revision 41
# baseline (speedup 1.0000x reference)
"""Multi-head attention (B=8, N=1024, D=768, H=12) on 8 Trainium2 NeuronCores.

Sharding: pure data parallel — one batch element per core, weights replicated,
no collectives. Host-side prep: x pre-transposed + cast fp16, weights fp16.

v2 schedule: the ScalarE's exp stream (~100us of engine time) is the pacing
resource, so emission is pipelined to start it as early as possible and keep
it dense:

  startup   : DMAs, consts, qkT for head-pair 0 (j=0 and j=6), then all v
  attention : slots (j, nch) j-major; the next slot's first score pair is
              prefetched inside the m loop (baseline trick); remaining qkT
              j-tiles are dispensed as fillers (1 unit per 3 iters) so the
              PE absorbs them in the slack left by the exp pacing
  tail      : output projection (score-PSUM pool closed, 3-buffered proj
              accumulators) + per-tile DMA out

Differences vs v1 kernel:
  - qkT/v emission overlapped with attention (v1 serialized ~70us of qkv
    before the second exp could run)
  - v/proj biases folded into the PSUM->SBUF drains via pre-broadcast rows
    (kills 32 K=1 bias matmuls)
  - softmax normalization: sums rows are copied into a [2,512] tile, one
    reciprocal_approx_fast, then ONE K=2 fp32r matmul with a {0,1} selector
    lhsT broadcasts both heads' 1/sums into a [128,512] tile (v1: 6 DVE ops
    + 2 fp16 casts + 2 K=1 matmuls per slot)
  - av is drained to SBUF fp16 immediately (frees the PSUM bank for the next
    slot) and the bc-matmul + normalization multiply are deferred two
    iterations into the next slot so the PE never head-of-line blocks on the
    DVE reciprocal chain

All matmuls fp16 (PSUM accumulation fp32) except the broadcast matmul
(fp32r, K=2, N=512 — full rate at N>=256, avoids fp16 casts of the recip).
"""

import numpy as np

import concourse.bass as bass
import concourse.bacc as bacc
import concourse.tile as tile
from concourse import mybir
from concourse.bass_utils import run_bass_kernel_spmd

F32 = mybir.dt.float32
F32R = mybir.dt.float32r
F16 = mybir.dt.float16
EXP = mybir.ActivationFunctionType.Exp
ADD = mybir.AluOpType.add
BYPASS = mybir.AluOpType.bypass

B = 8
N = 1024
D = 768
H = 12
HD = 64
SCALE = HD ** -0.5
NT = N // 128       # 8 tiles along sequence
DT = D // 128       # 6 tiles along features
QKT = 2 * D // 128  # 12 q+k feature tiles


def build_nc() -> bass.Bass:
    nc = bacc.Bacc(None)
    xT_d = nc.dram_tensor("xT", [D, N], F16, kind="ExternalInput")
    wqkv_d = nc.dram_tensor("w_qkv", [D, 3 * D], F16, kind="ExternalInput")
    bqkv_d = nc.dram_tensor("b_qkv", [3 * D], F32, kind="ExternalInput")
    wproj_d = nc.dram_tensor("w_proj", [D, D], F16, kind="ExternalInput")
    bproj_d = nc.dram_tensor("b_proj", [D], F32, kind="ExternalInput")
    out_d = nc.dram_tensor("out", [N, D], F32, kind="ExternalOutput")

    with tile.TileContext(nc) as tc:
        _emit(nc, tc, xT_d, wqkv_d, bqkv_d, wproj_d, bproj_d, out_d)
    nc.compile()
    return nc


def _emit(nc, tc, xT_d, wqkv_d, bqkv_d, wproj_d, bproj_d, out_d):
    from contextlib import ExitStack

    with ExitStack() as ctx:
        const = ctx.enter_context(tc.tile_pool(name="const", bufs=1))
        sb = ctx.enter_context(tc.tile_pool(name="sb", bufs=1))
        rsp = ctx.enter_context(tc.tile_pool(name="rsp", bufs=2))
        # pt depth 16: buffers the avfold stream's lag behind the exp stream
        # (up to ~14 iters during the startup ramp) without stalling ScalarE
        ptp = ctx.enter_context(tc.tile_pool(name="ptp", bufs=16))
        obp = ctx.enter_context(tc.tile_pool(name="obp", bufs=3))

        # ---- input DMAs: small bias DMAs FIRST (the strided bq_col takes
        # ~10us; queued late it stalls the bias matmuls for ~20us) ----
        brow32 = const.tile([1, 3 * D], F32, tag="brow32")
        nc.sync.dma_start(out=brow32, in_=bqkv_d[None, :])
        bprow32 = const.tile([1, D], F32, tag="bprow32")
        nc.sync.dma_start(out=bprow32, in_=bproj_d[None, :])
        # b_{q,k} partition-major for the per-partition bias add fused into
        # the qkT drains.
        bq_col = const.tile([128, QKT], F32, tag="bq_col")
        nc.sync.dma_start(
            out=bq_col,
            in_=bqkv_d[: 2 * D].rearrange("(j p) -> p j", p=128),
        )
        # DMA priority order (HBM is shared by all 8 cores -- effective
        # bandwidth makes the full 5.1MB take ~30us, so fetch by need):
        # xT + the startup sweep's qk column slices, then v columns, then
        # the remaining qk columns in filler consumption order, wproj last.
        xT = [sb.tile([128, N], F16, tag=f"xT{t}", name=f"xT{t}")
              for t in range(DT)]
        wqkv_sb = [sb.tile([128, 3 * D], F16, tag=f"wqkv{t}",
                           name=f"wqkv{t}") for t in range(DT)]
        for t in range(DT):
            nc.sync.dma_start(out=xT[t], in_=xT_d[128 * t:128 * (t + 1), :])
            for jt in (0, DT):
                nc.sync.dma_start(
                    out=wqkv_sb[t][:, 128 * jt:128 * (jt + 1)],
                    in_=wqkv_d[128 * t:128 * (t + 1),
                               128 * jt:128 * (jt + 1)])
        for t in range(DT):
            nc.sync.dma_start(
                out=wqkv_sb[t][:, 2 * D:3 * D],
                in_=wqkv_d[128 * t:128 * (t + 1), 2 * D:3 * D])
        for jt in [p_ for p in range(1, DT) for p_ in (p, DT + p)]:
            for t in range(DT):
                nc.sync.dma_start(
                    out=wqkv_sb[t][:, 128 * jt:128 * (jt + 1)],
                    in_=wqkv_d[128 * t:128 * (t + 1),
                               128 * jt:128 * (jt + 1)])
        brow = const.tile([1, 3 * D], F16, tag="brow")
        nc.vector.tensor_copy(brow, brow32)
        bprow = const.tile([1, D], F16, tag="bprow")
        nc.vector.tensor_copy(bprow, bprow32)

        ones_f32 = const.tile([128, 128], F32, tag="ones_f32")
        nc.vector.memset(ones_f32, 1.0)
        ones128 = const.tile([1, 128], F16, tag="ones128")
        nc.vector.tensor_copy(ones128, ones_f32[0:1, :])
        zbias = const.tile([128, 1], F32, tag="zbias")
        nc.vector.memset(zbias, 0.0)
        # dummy exp so the ~1.3us ACT_TABLE_LOAD happens during the DMA
        # window instead of delaying the first real exp
        dummy = const.tile([1, 1], F32, tag="dummy")
        nc.scalar.activation(dummy, zbias[0:1, 0:1], EXP, bias=zbias[0:1, :],
                             scale=1.0)

        wproj_sb = [sb.tile([128, D], F16, tag=f"wproj{t}",
                            name=f"wproj{t}") for t in range(DT)]
        for t in range(DT):
            nc.sync.dma_start(out=wproj_sb[t],
                              in_=wproj_d[128 * t:128 * (t + 1), :])

        qkT = [sb.tile([128, N], F16, tag=f"qkT{j}", name=f"qkT{j}")
               for j in range(QKT)]
        # va: per key tile, 12 heads x [v_h 64 | ones] = 780 cols. The ones
        # column rides along in the AV matmul (lhsT [128, 65]) so the softmax
        # denominators come out as row 64 of the AV psum -- no sums matmuls.
        va = [sb.tile([128, 65 * H], F16, tag=f"va{i}", name=f"va{i}")
              for i in range(NT)]
        waT = [sb.tile([128, N], F16, tag=f"waT{j}", name=f"waT{j}")
               for j in range(DT)]
        bv_bcast = const.tile([128, D], F16, tag="bv_bcast")
        bp_bcast = const.tile([128, D], F32, tag="bp_bcast")

        # ---- PSUM pools ----
        # startup: pboot takes all 8 banks for the boot sweep; it closes
        # before the attention pools open (pav 2 + pbc 1 + pacc 1 + pst 4).
        pboot_ctx = tc.tile_pool(name="pboot", bufs=8, space="PSUM")
        pboot = pboot_ctx.__enter__()

        # ---- work-unit emitters ----
        def emit_qk_unit(jt, nch, pool):
            """qkT feature tile jt, 512-query chunk nch: 6 matmuls + drain."""
            n0 = 512 * nch
            ps = pool.tile([128, 512], F32, tag="acc")
            for t in range(DT):
                nc.tensor.matmul(
                    ps,
                    wqkv_sb[t][:, 128 * jt:128 * (jt + 1)],
                    xT[t][:, n0:n0 + 512],
                    start=(t == 0), stop=(t == DT - 1),
                )
            nc.vector.tensor_scalar_add(
                qkT[jt][:, n0:n0 + 512], ps, bq_col[:, jt:jt + 1])

        def emit_v_unit(i, c0, cw, pool):
            """v token tile i, feature cols [c0, c0+cw): 6 matmuls + drain
            with the bias row pre-broadcast (no K=1 bias matmul). The drain
            scatters the heads into va's 65-col strides (col 0 = ones)."""
            h0, nh = c0 // HD, cw // HD
            ps = pool.tile([128, 512], F32, tag="acc")
            for t in range(DT):
                nc.tensor.matmul(
                    ps[:, 0:cw],
                    xT[t][:, 128 * i:128 * (i + 1)],
                    wqkv_sb[t][:, 2 * D + c0:2 * D + c0 + cw],
                    start=(t == 0), stop=(t == DT - 1),
                )
            va3 = va[i].rearrange("p (h c) -> p h c", c=65)
            nc.vector.scalar_tensor_tensor(
                va3[:, h0:h0 + nh, 0:64], ps[:, 0:cw], 1.0,
                bv_bcast[:, c0:c0 + cw], BYPASS, ADD)
            nc.vector.tensor_copy(va3[:, h0:h0 + nh, 64:65], ones_f32[:, 0:nh])

        # ---- PE warmup: ~3.5us of dummy matmuls so the HAM clock-gate is
        # at 8/8 before the real sweep starts (cold matmuls run at half
        # clock; the sweep would otherwise pay the whole ramp) ----
        wsrc32 = const.tile([1, 512], F32, tag="wsrc32")
        nc.vector.memset(wsrc32, 1.0)
        wsrc = const.tile([1, 512], F16, tag="wsrc")
        nc.vector.tensor_copy(wsrc, wsrc32)
        warm = pboot.tile([1, 512], F32, tag="acc")
        for _ in range(8):
            nc.tensor.matmul(warm, ones128[:, 0:1], wsrc,
                             start=True, stop=True)

        # ---- bias broadcast rows (4 tiny matmul+drain pairs) ----
        for c0, cw in ((0, 512), (512, 256)):
            ps = pboot.tile([128, 512], F32, tag="acc")
            nc.tensor.matmul(ps[:, 0:cw], ones128,
                             brow[:, 2 * D + c0:2 * D + c0 + cw],
                             start=True, stop=True)
            nc.vector.tensor_copy(bv_bcast[:, c0:c0 + cw], ps[:, 0:cw])
        for c0, cw in ((0, 512), (512, 256)):
            ps = pboot.tile([128, 512], F32, tag="acc")
            nc.tensor.matmul(ps[:, 0:cw], ones128,
                             bprow[:, c0:c0 + cw], start=True, stop=True)
            nc.vector.tensor_copy(bp_bcast[:, c0:c0 + cw], ps[:, 0:cw])

        # ---- startup sweep: qkT for head pair 0 swept t-major through 4
        # accumulators so the PE tracks the x/w DMA arrivals instead of
        # serializing after them; the exp stream starts right after. v would
        # serialize the sweep on its (later) column DMAs, so it streams into
        # slot 0 behind the lagged avfold instead. ----
        sweep_qk = [(0, 0), (0, 1), (DT, 0), (DT, 1)]
        qaccs = [pboot.tile([128, 512], F32, tag="acc", name=f"qacc{u}")
                 for u in range(len(sweep_qk))]
        for t in range(DT):
            for u, (jt, nch) in enumerate(sweep_qk):
                nc.tensor.matmul(
                    qaccs[u],
                    wqkv_sb[t][:, 128 * jt:128 * (jt + 1)],
                    xT[t][:, 512 * nch:512 * (nch + 1)],
                    start=(t == 0), stop=(t == DT - 1),
                )
        for u, (jt, nch) in enumerate(sweep_qk):
            nc.vector.tensor_scalar_add(
                qkT[jt][:, 512 * nch:512 * (nch + 1)], qaccs[u],
                bq_col[:, jt:jt + 1])
        pboot_ctx.__exit__(None, None, None)

        pav = ctx.enter_context(tc.tile_pool(name="pav", bufs=1, space="PSUM"))
        pbc = ctx.enter_context(tc.tile_pool(name="pbc", bufs=1, space="PSUM"))
        pacc = ctx.enter_context(
            tc.tile_pool(name="pacc", bufs=1, space="PSUM"))
        pst_ctx = tc.tile_pool(name="pst", bufs=2, space="PSUM")
        pst = pst_ctx.__enter__()

        # fillers dispensed into the attention iters as 3-matmul half-units
        # (a full 6-matmul unit in front of a score pair delays the exp
        # stream ~0.7us; halves cut the worst-case in-order blockage),
        # paced to track their weight-column DMA arrivals
        v_fill = [(i, c0, cw) for i in range(NT)
                  for c0, cw in ((0, 512), (512, 256))]
        qk_fill = [(jt, nch)
                   for p in range(1, DT)
                   for jt, nch in ((p, 0), (DT + p, 0), (DT + p, 1), (p, 1))]

        def gen_v_halves():
            for i, c0, cw in v_fill:
                h0, nh = c0 // HD, cw // HD
                ps = pacc.tile([128, 512], F32, tag="acc", name="ps")
                for t in range(DT):
                    nc.tensor.matmul(
                        ps[:, 0:cw],
                        xT[t][:, 128 * i:128 * (i + 1)],
                        wqkv_sb[t][:, 2 * D + c0:2 * D + c0 + cw],
                        start=(t == 0), stop=(t == DT - 1),
                    )
                    if t == 2:
                        yield
                va3 = va[i].rearrange("p (h c) -> p h c", c=65)
                nc.vector.scalar_tensor_tensor(
                    va3[:, h0:h0 + nh, 0:64], ps[:, 0:cw], 1.0,
                    bv_bcast[:, c0:c0 + cw], BYPASS, ADD)
                nc.vector.tensor_copy(va3[:, h0:h0 + nh, 64:65],
                                      ones_f32[:, 0:nh])
                yield

        def gen_qk_halves():
            for jt, nch in qk_fill:
                n0 = 512 * nch
                ps = pacc.tile([128, 512], F32, tag="acc", name="ps")
                for t in range(DT):
                    nc.tensor.matmul(
                        ps,
                        wqkv_sb[t][:, 128 * jt:128 * (jt + 1)],
                        xT[t][:, n0:n0 + 512],
                        start=(t == 0), stop=(t == DT - 1),
                    )
                    if t == 2:
                        yield
                nc.vector.tensor_scalar_add(
                    qkT[jt][:, n0:n0 + 512], ps, bq_col[:, jt:jt + 1])
                yield

        v_gen = gen_v_halves()
        qk_gen = gen_qk_halves()

        # ---- attention ----
        def emit_st(j, nch, m):
            """Row-packed K=64 pair: scores^T for heads (2j, 2j+1)."""
            m0, n0 = 128 * m, 512 * nch
            st = pst.tile([128, 1024], F32, tag="st")
            nc.tensor.matmul(
                st[:, 0:512],
                qkT[DT + j][0:64, m0:m0 + 128],
                qkT[j][0:64, n0:n0 + 512],
                start=True, stop=True,
            )
            nc.tensor.matmul(
                st[:, 512:1024],
                qkT[DT + j][64:128, m0:m0 + 128],
                qkT[j][64:128, n0:n0 + 512],
                start=True, stop=True,
            )
            return st

        # ---- flat attention pipeline ----
        # The exp stream runs at its own pace from ~13us; the avfold stream
        # LAGS it (ramping from 14 iters down to 2) so that v/qkT fillers --
        # themselves paced by their weight DMAs -- never sit between a score
        # pair and the exp that needs it. pt tiles buffer the lag.
        slots = [(j, nch) for j in range(DT) for nch in range(2)]
        iters = [(si, m) for si in range(len(slots)) for m in range(NT)]
        st_next = emit_st(*slots[0], 0)
        pts = {}       # iter index -> pt tile awaiting its avfold
        av_t = {}      # si -> (avA, avB)
        norms = []     # (due iter, deferred bc-matmul + normalize closure)
        cur = [0]

        def emit_tail(si2, avA, avB):
            """Post-slot drain: av16 + sums rows, reciprocal, fp16 casts;
            the bc matmul + normalize multiply are deferred two iters."""
            j2, nch2 = slots[si2]
            n02 = 512 * nch2
            av16 = rsp.tile([128, 512], F16, tag="av16")
            sa0 = rsp.tile([1, 512], F32, tag="sa0")
            sb0 = rsp.tile([1, 512], F32, tag="sb0")
            nc.vector.tensor_copy(av16[0:64, :], avA[0:64, :])
            nc.vector.tensor_copy(sa0, avA[64:65, :])
            nc.vector.tensor_copy(av16[64:128, :], avB[0:64, :])
            nc.vector.tensor_copy(sb0, avB[64:65, :])
            ra32 = rsp.tile([1, 512], F32, tag="ra32")
            rb32 = rsp.tile([1, 512], F32, tag="rb32")
            nc.vector.reciprocal_approx_fast(ra32, sa0)
            nc.vector.reciprocal_approx_fast(rb32, sb0)
            ra = rsp.tile([1, 512], F16, tag="ra")
            rb = rsp.tile([1, 512], F16, tag="rb")
            nc.vector.tensor_copy(ra, ra32)
            nc.vector.tensor_copy(rb, rb32)

            def norm():
                # broadcast 1/sums to 64 partitions per head via K=1
                # matmuls, col-packed (0,0)/(0,64) into one bank
                bc = pbc.tile([128, 512], F32, tag="bc")
                nc.tensor.matmul(bc[0:64, :], ones128[:, 0:64], ra,
                                 start=True, stop=True)
                nc.tensor.matmul(bc[64:128, :], ones128[:, 0:64], rb,
                                 start=True, stop=True,
                                 skip_group_check=True)
                nc.vector.tensor_mul(waT[j2][:, n02:n02 + 512], av16, bc)

            norms.append((cur[0] + 2, norm))

        def emit_avfold(k2):
            si2, m2 = iters[k2]
            j2, _ = slots[si2]
            hA, hB = 2 * j2, 2 * j2 + 1
            if m2 == 0:
                av_t[si2] = (pav.tile([128, 512], F32, tag="avA", name="avA"),
                             pav.tile([128, 512], F32, tag="avB", name="avB"))
            avA, avB = av_t[si2]
            pt_t = pts.pop(k2)
            # attention output with the ones column folded in: row 64 of
            # each av accumulates the softmax denominators, rows 0:64 the
            # unnormalized head output.
            nc.tensor.matmul(
                avA[0:65, :], va[m2][:, 65 * hA:65 * hA + 65],
                pt_t[:, 0:512], start=(m2 == 0), stop=(m2 == NT - 1),
            )
            nc.tensor.matmul(
                avB[0:65, :], va[m2][:, 65 * hB:65 * hB + 65],
                pt_t[:, 512:1024], start=(m2 == 0), stop=(m2 == NT - 1),
            )
            if m2 == NT - 1:
                emit_tail(si2, avA, avB)

        av_done = 0
        for k, (si, m) in enumerate(iters):
            cur[0] = k
            st = st_next
            pt_t = ptp.tile([128, 1024], F16, tag="pt")
            pts[k] = pt_t
            nc.scalar.activation(pt_t, st, EXP, bias=zbias, scale=SCALE)
            if k + 1 < len(iters):
                si2, m2 = iters[k + 1]
                st_next = emit_st(*slots[si2], m2)
            # fillers, paced to their DMA arrivals
            if 5 <= k <= 15:
                for _ in range(3):
                    next(v_gen, None)
            if k >= 13:
                next(qk_gen, None)
                next(qk_gen, None)
            # lagged avfold stream: ramp from lag ~14 to steady lag 2
            want = max(0, min(k - 1, 2 * (k - 13)))
            while av_done < want:
                emit_avfold(av_done)
                av_done += 1
            while norms and norms[0][0] <= k:
                norms.pop(0)[1]()
        while av_done < len(iters):
            emit_avfold(av_done)
            av_done += 1
        while len(norms) > 1:
            norms.pop(0)[1]()
        pending_norm = norms.pop(0)[1]
        pst_ctx.__exit__(None, None, None)

        # ---- output projection (score PSUM banks now free) ----
        # the deferred norm of the last slot only gates waT[5][:, 512:1024],
        # i.e. proj n-tiles 4..7 -- flush it under n-tile 0's matmuls
        with tc.tile_pool(name="po", bufs=3, space="PSUM") as po:
            for i in range(NT):
                ot = obp.tile([128, D], F32, tag="ot")
                for c0, cw in ((0, 512), (512, 256)):
                    ps = po.tile([128, 512], F32, tag="o")
                    for t in range(DT):
                        nc.tensor.matmul(
                            ps[:, 0:cw],
                            waT[t][:, 128 * i:128 * (i + 1)],
                            wproj_sb[t][:, c0:c0 + cw],
                            start=(t == 0), stop=(t == DT - 1),
                        )
                    nc.vector.scalar_tensor_tensor(
                        ot[:, c0:c0 + cw], ps[:, 0:cw], 1.0,
                        bp_bcast[:, c0:c0 + cw], BYPASS, ADD)
                if i == 0:
                    pending_norm()
                nc.sync.dma_start(out=out_d[128 * i:128 * (i + 1), :], in_=ot)


def run(inputs: dict, trace: bool = False):
    """Build, compile and run on all 8 cores. Returns (out [B,N,D], results)."""
    nc = build_nc()
    x = np.asarray(inputs["x"], dtype=np.float32)
    shared = {
        "w_qkv": np.asarray(inputs["w_qkv"], dtype=np.float32).astype(np.float16),
        "b_qkv": np.asarray(inputs["b_qkv"], dtype=np.float32),
        "w_proj": np.asarray(inputs["w_proj"], dtype=np.float32).astype(np.float16),
        "b_proj": np.asarray(inputs["b_proj"], dtype=np.float32),
    }
    in_maps = [
        {"xT": np.ascontiguousarray(x[b].T.astype(np.float16)), **shared}
        for b in range(B)
    ]
    res = run_bass_kernel_spmd(nc, in_maps, list(range(B)), trace=trace)
    out = np.stack([res.results[b]["out"] for b in range(B)], axis=0)
    return out, res


def kernel(x, w_qkv, b_qkv, w_proj, b_proj) -> np.ndarray:
    out, _ = run(
        {"x": x, "w_qkv": w_qkv, "b_qkv": b_qkv, "w_proj": w_proj,
         "b_proj": b_proj}
    )
    return out
